# revision 39
# baseline (speedup 1.0000x reference)
"""BrainGAT (2x GATv2Conv + residuals + FC) on 8 Trainium2 NeuronCores.

Sharding: nodes partitioned across 8 cores via a load-balanced permutation
(160 bins of 128 slots, 125 real nodes each, in-degree balanced by snake
dealing); edges assigned to the bin owning their destination. Small weights
replicated.

Layer 1 needs no device gather at all: the host pre-gathers raw source
features per edge slot (XselT, [128ch x 128e] per tile, bf16) and the
device computes per-edge xl via XselT @ Wl1T on the Tensor engine -- the
same matmul slot the identity expansion used to occupy -- so the Q7 SWDGE
descriptor loop (~9ns/row) and the first AllGather disappear. The host
also supplies the dst one-hot matrices oh/ohT directly (DVE is_equal
builds removed). Layer 2 still gathers xl2 rows from the AllGathered
table, but as one 2048-idx single_packet=False gather per block spread
over 4 SWDGE queues (overlapping Q7 generation, ~3.9ns/idx).

Per-edge exp-denominator is fused into the numerator scatter (pexp rides
as 4 extra wptn columns through the same PSUM accumulation). Segment
softmax uses the exact no-max rewrite: alpha_e = exp(logit_e) /
(sum exp + exp(m_d)) with m_d the self-loop logit, so self-loops never
enter the edge lists. Linear biases bl ride through the softmax
(sum alpha = 1) and are folded into the xr-side and output biases.
"""
import numpy as np

import concourse.bass as bass
import concourse.bacc as bacc
import concourse.mybir as mybir
import concourse.tile as tile
from concourse.bass_utils import run_bass_kernel_spmd

f32 = mybir.dt.float32
bf16 = mybir.dt.bfloat16
i16 = mybir.dt.int16
NPBF = mybir.dt.np(bf16)
AF = mybir.ActivationFunctionType
ALU = mybir.AluOpType
PRELU = AF.Prelu

NC = 8
HEADS = 4
NEG_SLOPE = 0.2
P = 128
B = 128            # dst-block size (slots per bin)
NB = 20            # blocks per core
NPC = B * NB       # node slots per core (2560; 2500 real)
NREAL = 2500       # real nodes per core
NFULL = NC * NPC   # padded global table rows
IN_CH = 128
HC1 = 256
HC2 = 128
OUT_CH = 64


# ----------------------------------------------------------------------------
# device program
# ----------------------------------------------------------------------------

def build_program(NT, dbg=False):
    EB = NT * P               # edges per block (padded)
    IC = EB // 16             # idx cols per block (L2 gather)
    nc = bacc.Bacc("TRN2", target_bir_lowering=False, debug=False,
                   num_swdge_queues=4)

    def inp(name, shape, dt=f32):
        return nc.dram_tensor(name, shape, dt, kind="ExternalInput")

    xT = inp("xT", [IN_CH, NPC], bf16)
    xselT = inp("xselT", [P, NB * EB], bf16)      # [128ch, (b t q)]
    ohT_in = inp("ohT_in", [P, NB * EB], bf16)    # [slot p, (b t q)]
    oh_in = inp("oh_in", [P, NB * EB], bf16)      # [edge p, (b t q)]
    src_idx = inp("src_idx", [P, NB * IC], i16)   # L2 gather gids

    Wl1T = inp("Wl1T", [IN_CH, HC1], bf16); Wr1T = inp("Wr1T", [IN_CH, HC1], bf16)
    P1T = inp("P1T", [IN_CH, HC1], bf16)
    brl1_bc = inp("brl1_bc", [P, HC1])            # br1 + bl1
    pb1m1_bc = inp("pb1m1_bc", [P, HC1])          # pb1 - 1
    bias1_bc = inp("bias1_bc", [P, HC1])          # bias1 + bl1
    att1_bc = inp("att1_bc", [P, HC1], bf16)
    att1_rep = inp("att1_rep", [P, NT * HC1], bf16)

    Wl2T = inp("Wl2T", [HC1, HC2], bf16); Wr2T = inp("Wr2T", [HC1, HC2], bf16)
    P2T = inp("P2T", [HC1, HC2], bf16)
    brl2_bc = inp("brl2_bc", [P, HC2])            # br2 + bl2
    pb2m1_bc = inp("pb2m1_bc", [P, HC2])          # pb2 - 1
    bias2_bc = inp("bias2_bc", [P, HC2])          # bias2 + bl2
    att2_bc = inp("att2_bc", [P, HC2], bf16)
    att2_rep = inp("att2_rep", [P, NT * HC2], bf16)

    WfT = inp("WfT", [HC2, OUT_CH], bf16); bf_bc = inp("bf_bc", [P, OUT_CH])

    CHK = 5  # blocks per AllGather chunk
    NCHK = NB // CHK
    xl2_own = nc.dram_tensor("xl2_own", [NPC, HC2], bf16)
    # chunk-major full table: [chunk, core * (CHK*B) + row, ch] so each
    # chunk's AllGather output slice is contiguous
    xl2_full = nc.dram_tensor("xl2_full", [NCHK, NC * CHK * B, HC2], bf16,
                              addr_space="Shared")
    out_own = nc.dram_tensor("out_own", [NPC, OUT_CH], f32,
                             kind="ExternalOutput")
    if dbg:
        out_dbg = nc.dram_tensor("out_dbg", [P, 6 * HC1 + HC1 + HEADS], f32,
                                 kind="ExternalOutput")

    from concourse.masks import make_identity

    with tile.TileContext(nc) as tc:
        with (
            tc.tile_pool(name="const", bufs=1) as cp,
            tc.tile_pool(name="res", bufs=1) as rp,
            tc.tile_pool(name="work", bufs=2) as wp,
            tc.tile_pool(name="gath", bufs=4) as gp,
            tc.tile_pool(name="onehot", bufs=2) as op,
        ):
            # ---- constants
            identb = cp.tile([P, P], bf16)
            make_identity(nc, identb[:])
            with tc.tile_pool(name="pwarm", bufs=1, space="PSUM") as pw:
                warm = pw.tile([P, P], bf16)
                nc.tensor.transpose(out=warm[:], in_=identb[:],
                                    identity=identb[:])

            def load_const(t, shape):
                s = cp.tile(shape, t.dtype, tag=f"c_{t.name}")
                nc.sync.dma_start(out=s[:], in_=t[:, :])
                return s

            w1 = {k: load_const(v, [IN_CH, HC1])
                  for k, v in (("wl", Wl1T), ("wr", Wr1T), ("p", P1T))}
            c1b = {k: load_const(v, [P, HC1]) for k, v in (
                ("brl", brl1_bc), ("pbm1", pb1m1_bc), ("bias", bias1_bc))}
            c1b["attr"] = load_const(att1_rep, [P, NT * HC1])
            c1b["att"] = c1b["attr"][:, 0:HC1]

            def load_w2(t):
                s = cp.tile([P, 2 * HC2], t.dtype, tag=f"c_{t.name}")
                for c in range(2):
                    nc.sync.dma_start(out=s[:, c * HC2:(c + 1) * HC2],
                                      in_=t[c * P:(c + 1) * P, :])
                return s

            w2 = {k: load_w2(v) for k, v in (("wl", Wl2T), ("wr", Wr2T),
                                             ("p", P2T))}
            c2b = {k: load_const(v, [P, HC2]) for k, v in (
                ("brl", brl2_bc), ("pbm1", pb2m1_bc), ("bias", bias2_bc))}
            c2b["attr"] = load_const(att2_rep, [P, NT * HC2])
            c2b["att"] = c2b["attr"][:, 0:HC2]
            wf_sb = load_const(WfT, [HC2, OUT_CH])
            bf_sb = load_const(bf_bc, [P, OUT_CH])

            xT_sb = rp.tile([IN_CH, NPC], bf16)
            nc.sync.dma_start(out=xT_sb[:], in_=xT[:, :])
            sidx_sb = rp.tile([P, NB * IC], i16)
            nc.sync.dma_start(out=sidx_sb[:], in_=src_idx[:, :])

            # ---- persistent per-layer node tensors
            xl1_sb = rp.tile([P, NB * HC1], bf16)   # no bl1
            xr1_sb = rp.tile([P, NB * HC1], bf16)   # + br1 + bl1
            id1_sb = rp.tile([P, NB * HC1], bf16)
            em1_sb = rp.tile([P, NB * HEADS], f32)  # exp(self logit)
            h_sb = rp.tile([P, NB * HC1], bf16)
            hT_sb = rp.tile([P, NB * HC1], bf16)
            xl2_sb = rp.tile([P, NB * HC2], bf16)   # no bl2
            xr2_sb = rp.tile([P, NB * HC2], bf16)   # + br2 + bl2
            id2_sb = rp.tile([P, NB * HC2], bf16)
            em2_sb = rp.tile([P, NB * HEADS], f32)
            h2_sb = rp.tile([P, NB * HC2], bf16)

            # ---- N1: xl (no bias), xr (+brl), id, self-logit exp
            with tc.tile_pool(name="pn1", bufs=2, space="PSUM") as ps:
                for b in range(NB):
                    lhsT = xT_sb[:, b * B:(b + 1) * B]
                    pxl = ps.tile([P, HC1], f32, tag="pxl")
                    pxr = ps.tile([P, HC1], f32, tag="pxr")
                    pid = ps.tile([P, HC1], f32, tag="pid")
                    nc.tensor.matmul(out=pxl[:], lhsT=lhsT, rhs=w1["wl"][:],
                                     start=True, stop=True)
                    nc.tensor.matmul(out=pxr[:], lhsT=lhsT, rhs=w1["wr"][:],
                                     start=True, stop=True)
                    nc.tensor.matmul(out=pid[:], lhsT=lhsT, rhs=w1["p"][:],
                                     start=True, stop=True)
                    xl_t = xl1_sb[:, b * HC1:(b + 1) * HC1]
                    xr_t = xr1_sb[:, b * HC1:(b + 1) * HC1]
                    nc.scalar.activation(xl_t, pxl[:], AF.Copy)
                    nc.vector.tensor_add(out=xr_t, in0=pxr[:], in1=c1b["brl"][:])
                    nc.vector.tensor_add(out=id1_sb[:, b * HC1:(b + 1) * HC1],
                                         in0=pid[:], in1=c1b["pbm1"][:])
                    t0 = wp.tile([P, HC1], bf16, tag="t0")
                    nc.vector.tensor_add(out=t0[:], in0=xl_t, in1=xr_t)
                    nc.scalar.activation(t0[:], t0[:], PRELU, alpha=NEG_SLOPE)
                    nc.vector.tensor_mul(out=t0[:], in0=t0[:], in1=c1b["att"])
                    em_t = em1_sb[:, b * HEADS:(b + 1) * HEADS]
                    nc.vector.reduce_sum(
                        out=em_t,
                        in_=t0[:].rearrange("p (h c) -> p h c", h=HEADS),
                        axis=mybir.AxisListType.X)
                    nc.scalar.activation(em_t, em_t, AF.Exp)

            # ---- generic finalize: seg psum [P, hc+4] -> h_out block
            # xr_sub: when seg accumulated pexp*tt (tt = xl+xr_dst) instead of
            # pexp*xl, subtract xr_d * segE_d (exact: all edges share dst d).
            def finalize(b, segp, xl_sb, id_sb, em_sb, hout_sb, hc, consts,
                         xr_sub=None):
                CH = hc // HEADS
                e4 = em_sb[:, b * HEADS:(b + 1) * HEADS]
                den = wp.tile([P, HEADS], f32, tag="den")
                nc.vector.tensor_add(out=den[:], in0=segp[:, hc:hc + HEADS],
                                     in1=e4)
                rec = wp.tile([P, HEADS], f32, tag="rec")
                nc.vector.reciprocal(out=rec[:], in_=den[:])
                num = wp.tile([P, hc], f32, tag="num")
                nc.vector.tensor_tensor(
                    out=num[:].rearrange("p (h c) -> p h c", h=HEADS),
                    in0=xl_sb[:, b * hc:(b + 1) * hc]
                        .rearrange("p (h c) -> p h c", h=HEADS),
                    in1=e4.unsqueeze(2).to_broadcast([P, HEADS, CH]),
                    op=ALU.mult)
                nc.vector.tensor_add(out=num[:], in0=num[:], in1=segp[:, 0:hc])
                if xr_sub is not None:
                    t2 = wp.tile([P, hc], f32, tag="t2")
                    nc.vector.tensor_tensor(
                        out=t2[:].rearrange("p (h c) -> p h c", h=HEADS),
                        in0=xr_sub.rearrange("p (h c) -> p h c", h=HEADS),
                        in1=segp[:, hc:hc + HEADS].unsqueeze(2)
                            .to_broadcast([P, HEADS, CH]),
                        op=ALU.mult)
                    nc.vector.tensor_tensor(out=num[:], in0=num[:], in1=t2[:],
                                            op=ALU.subtract)
                nc.vector.tensor_tensor(
                    out=num[:].rearrange("p (h c) -> p h c", h=HEADS),
                    in0=num[:].rearrange("p (h c) -> p h c", h=HEADS),
                    in1=rec[:].unsqueeze(2).to_broadcast([P, HEADS, CH]),
                    op=ALU.mult)
                nc.vector.tensor_add(out=num[:], in0=num[:],
                                     in1=consts["bias"][:])
                # h = elu(num) + id = max(num,0) + (min(exp(num),1) + id)
                eu = wp.tile([P, hc], f32, tag="eu")
                nc.scalar.activation(eu[:], num[:], AF.Exp)
                t1 = wp.tile([P, hc], f32, tag="t1")
                nc.vector.scalar_tensor_tensor(
                    out=t1[:], in0=eu[:], scalar=1.0,
                    in1=id_sb[:, b * hc:(b + 1) * hc],
                    op0=ALU.min, op1=ALU.add)
                nc.vector.scalar_tensor_tensor(
                    out=hout_sb[:, b * hc:(b + 1) * hc], in0=num[:],
                    scalar=0.0, in1=t1[:], op0=ALU.max, op1=ALU.add)

            # ---- shared edge-phase tail: logits -> pexp -> wptn -> seg
            def edge_tail(b, tb, xlg, hc, consts, psa, att_eng=None):
                lgf = wp.tile([P, NT * HEADS], f32, tag="lgf")
                (att_eng or nc.vector).tensor_mul(
                    out=tb[:].rearrange("p t e -> p (t e)"),
                    in0=tb[:].rearrange("p t e -> p (t e)"),
                    in1=consts["attr"][:])
                nc.vector.reduce_sum(
                    out=lgf[:].rearrange("p (t h) -> p t h", t=NT),
                    in_=tb[:].rearrange("p t (h c) -> p t h c", h=HEADS),
                    axis=mybir.AxisListType.X)
                wptn = wp.tile([P, NT, hc + HEADS], bf16, tag="wptn")
                nc.scalar.activation(
                    wptn[:, :, hc:hc + HEADS],
                    lgf[:].rearrange("p (t h) -> p t h", t=NT), AF.Exp)
                nc.vector.tensor_tensor(
                    out=wptn[:, :, 0:hc]
                        .rearrange("p t (h c) -> p t h c", h=HEADS),
                    in0=xlg[:].rearrange("p t (h c) -> p t h c", h=HEADS),
                    in1=wptn[:, :, hc:hc + HEADS]
                        .unsqueeze(3).to_broadcast([P, NT, HEADS, hc // HEADS]),
                    op=ALU.mult)
                oh_sb = op.tile([P, NT, P], bf16, tag="oh")
                nc.sync.dma_start(
                    out=oh_sb[:].rearrange("p t q -> p (t q)"),
                    in_=oh_in[:, b * EB:(b + 1) * EB])
                seg = psa.tile([P, hc + HEADS], f32, tag="seg")
                for k in range(NT):
                    nc.tensor.matmul(out=seg[:], lhsT=oh_sb[:, k, :],
                                     rhs=wptn[:, k, :],
                                     start=(k == 0), stop=(k == NT - 1))
                return seg

            # ---- layer-1 edge phase (no gathers: XselT @ Wl1T on PE)
            with (
                tc.tile_pool(name="pe1", bufs=2, space="PSUM") as psa,
                tc.tile_pool(name="pt1", bufs=6, space="PSUM") as pst,
            ):
                for b in range(NB):
                    xsel_sb = gp.tile([P, NT, P], bf16, tag="gsrc")
                    nc.sync.dma_start(
                        out=xsel_sb[:].rearrange("p t q -> p (t q)"),
                        in_=xselT[:, b * EB:(b + 1) * EB])
                    ohT_sb = op.tile([P, NT, P], bf16, tag="ohT")
                    nc.sync.dma_start(
                        out=ohT_sb[:].rearrange("p t q -> p (t q)"),
                        in_=ohT_in[:, b * EB:(b + 1) * EB])
                    xrb = xr1_sb[:, b * HC1:(b + 1) * HC1]
                    tts = wp.tile([P, NT, HC1], bf16, tag="tts")
                    tb = wp.tile([P, NT, HC1], bf16, tag="tb")
                    for j in range(NT // 2):
                        ttp = pst.tile([P, 2, HC1], f32, tag="ttp")
                        for i in range(2):
                            k = 2 * j + i
                            nc.tensor.matmul(out=ttp[:, i, :],
                                             lhsT=ohT_sb[:, k, :], rhs=xrb,
                                             start=True, stop=False)
                            nc.tensor.matmul(out=ttp[:, i, :],
                                             lhsT=xsel_sb[:, k, :],
                                             rhs=w1["wl"][:],
                                             start=False, stop=True)
                        nc.scalar.activation(
                            tts[:, 2 * j:2 * j + 2, :]
                                .rearrange("p t e -> p (t e)"),
                            ttp[:].rearrange("p t e -> p (t e)"), AF.Copy)
                        nc.scalar.activation(
                            tb[:, 2 * j:2 * j + 2, :]
                                .rearrange("p t e -> p (t e)"),
                            ttp[:].rearrange("p t e -> p (t e)"),
                            PRELU, alpha=NEG_SLOPE)
                    seg = edge_tail(b, tb, tts, HC1, c1b, psa)
                    if dbg and b == 0:
                        nc.gpsimd.dma_start(out=out_dbg.ap()[:, 0:HC1],
                                            in_=xl1_sb[:, 0:HC1])
                        nc.gpsimd.dma_start(out=out_dbg.ap()[:, HC1:2 * HC1],
                                            in_=xr1_sb[:, 0:HC1])
                        nc.gpsimd.dma_start(out=out_dbg.ap()[:, 2 * HC1:3 * HC1],
                                            in_=tts[:, 0, :])
                        nc.gpsimd.dma_start(out=out_dbg.ap()[:, 3 * HC1:4 * HC1],
                                            in_=tb[:, 0, :])
                        dbgt = wp.tile([P, HC1 + 2 * HEADS], f32, tag="dbg")
                        nc.vector.tensor_copy(out=dbgt[:, 0:HC1 + HEADS],
                                              in_=seg[:])
                        nc.vector.tensor_copy(
                            out=dbgt[:, HC1 + HEADS:HC1 + 2 * HEADS],
                            in_=em1_sb[:, 0:HEADS])
                        nc.sync.dma_start(
                            out=out_dbg.ap()[:, 4 * HC1:5 * HC1 + 2 * HEADS],
                            in_=dbgt[:])
                    finalize(b, seg, xl1_sb, id1_sb, em1_sb, h_sb, HC1, c1b,
                             xr_sub=xrb)
                    if dbg and b == 0:
                        nc.gpsimd.dma_start(
                            out=out_dbg.ap()[:, 6 * HC1:7 * HC1],
                            in_=h_sb[:, 0:HC1])

            # ---- N2a: transpose h via HWDGE X-bar; N2b: xl2 (no bias)
            for b in range(NB):
                for c in range(2):
                    sl = slice(b * HC1 + c * P, b * HC1 + (c + 1) * P)
                    nc.sync.dma_start(out=hT_sb[:, sl], in_=h_sb[:, sl],
                                      transpose=True)
            with tc.tile_pool(name="pn2b", bufs=2, space="PSUM") as ps:
                for b in range(NB):
                    pxl = ps.tile([P, HC2], f32, tag="p2xl")
                    for c in range(2):
                        nc.tensor.matmul(
                            out=pxl[:],
                            lhsT=hT_sb[:, b * HC1 + c * P:b * HC1 + (c + 1) * P],
                            rhs=w2["wl"][:, c * HC2:(c + 1) * HC2],
                            start=(c == 0), stop=(c == 1))
                    nc.scalar.activation(xl2_sb[:, b * HC2:(b + 1) * HC2],
                                         pxl[:], AF.Copy)
            for c0 in range(NCHK):
                nc.sync.dma_start(
                    out=xl2_own.ap()[c0 * CHK * B:(c0 + 1) * CHK * B, :]
                        .rearrange("(b p) c -> p b c", p=B),
                    in_=xl2_sb[:, c0 * CHK * HC2:(c0 + 1) * CHK * HC2]
                        .rearrange("p (b c) -> p b c", b=CHK))
                nc.gpsimd.collective_compute(
                    "AllGather", ALU.bypass,
                    replica_groups=[list(range(NC))],
                    ins=[xl2_own.ap()[c0 * CHK * B:(c0 + 1) * CHK * B, :]
                         .opt()],
                    outs=[xl2_full.ap()[c0, :, :].opt()])
            with tc.tile_pool(name="pn2c", bufs=2, space="PSUM") as ps:
                for b in range(NB):
                    pxr = ps.tile([P, HC2], f32, tag="p2xr")
                    pid = ps.tile([P, HC2], f32, tag="p2id")
                    for c in range(2):
                        lhsT = hT_sb[:, b * HC1 + c * P:b * HC1 + (c + 1) * P]
                        st, sp = (c == 0), (c == 1)
                        cs = slice(c * HC2, (c + 1) * HC2)
                        nc.tensor.matmul(out=pxr[:], lhsT=lhsT,
                                         rhs=w2["wr"][:, cs], start=st, stop=sp)
                        nc.tensor.matmul(out=pid[:], lhsT=lhsT,
                                         rhs=w2["p"][:, cs], start=st, stop=sp)
                    xr_t = xr2_sb[:, b * HC2:(b + 1) * HC2]
                    nc.vector.tensor_add(out=xr_t, in0=pxr[:], in1=c2b["brl"][:])
                    nc.vector.tensor_add(out=id2_sb[:, b * HC2:(b + 1) * HC2],
                                         in0=pid[:], in1=c2b["pbm1"][:])
                    t0 = wp.tile([P, HC2], bf16, tag="t02")
                    nc.vector.tensor_add(out=t0[:],
                                         in0=xl2_sb[:, b * HC2:(b + 1) * HC2],
                                         in1=xr_t)
                    nc.scalar.activation(t0[:], t0[:], PRELU, alpha=NEG_SLOPE)
                    nc.vector.tensor_mul(out=t0[:], in0=t0[:], in1=c2b["att"])
                    em_t = em2_sb[:, b * HEADS:(b + 1) * HEADS]
                    nc.vector.reduce_sum(
                        out=em_t,
                        in_=t0[:].rearrange("p (h c) -> p h c", h=HEADS),
                        axis=mybir.AxisListType.X)
                    nc.scalar.activation(em_t, em_t, AF.Exp)

            # ---- layer-2 edge phase (gathered xlg; tt = ohT@xr2 + xlg on DVE)
            with (
                tc.tile_pool(name="pe2", bufs=2, space="PSUM") as psa,
                tc.tile_pool(name="pt2", bufs=4, space="PSUM") as pst,
            ):
                for b in range(NB):
                    xlg = gp.tile([P, NT, HC2], bf16, tag="gsrc")
                    nc.gpsimd.dma_gather(
                        xlg[:], xl2_full.ap().rearrange("a b c -> (a b) c"),
                        sidx_sb[:, b * IC:(b + 1) * IC],
                        EB, EB, HC2, single_packet=False, queue_num=b % 4)
                    ohT_sb = op.tile([P, NT, P], bf16, tag="ohT")
                    nc.sync.dma_start(
                        out=ohT_sb[:].rearrange("p t q -> p (t q)"),
                        in_=ohT_in[:, b * EB:(b + 1) * EB])
                    xrb = xr2_sb[:, b * HC2:(b + 1) * HC2]
                    tb = wp.tile([P, NT, HC2], bf16, tag="tb2")
                    for j in range(NT // 2):
                        ttp = pst.tile([P, 2, HC2], f32, tag="ttp2")
                        for i in range(2):
                            k = 2 * j + i
                            nc.tensor.matmul(out=ttp[:, i, :],
                                             lhsT=ohT_sb[:, k, :], rhs=xrb,
                                             start=True, stop=True)
                        nc.vector.tensor_add(
                            out=tb[:, 2 * j:2 * j + 2, :]
                                .rearrange("p t e -> p (t e)"),
                            in0=ttp[:].rearrange("p t e -> p (t e)"),
                            in1=xlg[:, 2 * j:2 * j + 2, :]
                                .rearrange("p t e -> p (t e)"))
                        nc.scalar.activation(
                            tb[:, 2 * j:2 * j + 2, :]
                                .rearrange("p t e -> p (t e)"),
                            tb[:, 2 * j:2 * j + 2, :]
                                .rearrange("p t e -> p (t e)"),
                            PRELU, alpha=NEG_SLOPE)
                    seg = edge_tail(b, tb, xlg, HC2, c2b, psa)
                    finalize(b, seg, xl2_sb, id2_sb, em2_sb, h2_sb, HC2, c2b)

            # ---- FC
            with tc.tile_pool(name="pfc", bufs=2, space="PSUM") as ps:
                for b in range(NB):
                    h2T = wp.tile([P, P], bf16, tag="fcT")
                    nc.sync.dma_start(out=h2T[:],
                                      in_=h2_sb[:, b * HC2:(b + 1) * HC2],
                                      transpose=True)
                    pf = ps.tile([P, OUT_CH], f32, tag="fc_out")
                    nc.tensor.matmul(out=pf[:], lhsT=h2T[:], rhs=wf_sb[:],
                                     start=True, stop=True)
                    ob = wp.tile([P, OUT_CH], f32, tag="fc_ob")
                    nc.vector.tensor_add(out=ob[:], in0=pf[:], in1=bf_sb[:])
                    nc.sync.dma_start(out=out_own.ap()[b * B:(b + 1) * B, :],
                                      in_=ob[:])
    nc.compile()
    return nc


# ----------------------------------------------------------------------------
# host-side sharding / input prep
# ----------------------------------------------------------------------------

def balanced_assignment(dst):
    """Snake-deal nodes (by in-degree desc) into NC*NB bins of NPC/NB slots."""
    N = NC * NREAL
    nbins = NC * NB
    rounds = N // nbins  # 125
    deg = np.bincount(dst, minlength=N)
    order = np.argsort(-deg, kind="stable")
    bin_of = np.empty(N, np.int32)
    slot_of = np.empty(N, np.int32)
    cols = np.arange(nbins)
    for r in range(rounds):
        nodes = order[r * nbins:(r + 1) * nbins]
        c = cols if r % 2 == 0 else cols[::-1]
        bin_of[nodes] = c
        slot_of[nodes] = r
    return bin_of, slot_of


def wrap_idx(vals, EB):
    """[EB] int -> [128, EB//16] int16 wrapped in 16 partitions, replicated."""
    w = np.zeros((16, EB // 16), np.int16)
    w[np.arange(EB) % 16, np.arange(EB) // 16] = vals.astype(np.int16)
    return np.tile(w, (8, 1))


def prep_inputs(x, edge_index, weights):
    src = np.asarray(edge_index[0], dtype=np.int64)
    dst = np.asarray(edge_index[1], dtype=np.int64)

    bin_of, slot_of = balanced_assignment(dst)
    core_of = bin_of // NB
    block_of = bin_of % NB
    # row in the chunk-major L2 full table [chunk, core*(CHK*B)+row, ch]
    CHK = 5
    gid = ((block_of // CHK) * (NC * CHK * B) + core_of * (CHK * B)
           + (block_of % CHK) * B + slot_of)

    ebin = bin_of[dst]
    counts = np.bincount(ebin, minlength=NC * NB)
    NT = max(1, int(np.ceil(counts.max() / P)))
    EB = NT * P
    IC = EB // 16

    eorder = np.argsort(ebin, kind="stable")
    offs = np.zeros(NC * NB + 1, np.int64)
    np.cumsum(counts, out=offs[1:])

    src_gid = gid[src]
    dst_slot = slot_of[dst]
    xf = np.asarray(x, np.float32)
    xT_bf = np.ascontiguousarray(xf.T).astype(NPBF)  # [128, N] by node id

    in_maps = []
    eye = np.eye(P, dtype=NPBF)
    for c in range(NC):
        sarr = np.zeros((P, NB * IC), np.int16)
        xsel = np.zeros((P, NB * EB), NPBF)
        ohT = np.zeros((P, NB * EB), NPBF)
        oh = np.zeros((P, NB * EB), NPBF)
        for b in range(NB):
            bi = c * NB + b
            eb = eorder[offs[bi]:offs[bi + 1]]
            n = len(eb)
            sv = np.zeros(EB, np.int64); sv[:n] = src_gid[eb]
            sarr[:, b * IC:(b + 1) * IC] = wrap_idx(sv, EB)
            # per-edge source features [128ch, EB]
            xsel[:, b * EB:b * EB + n] = xT_bf[:, src[eb]]
            # one-hot dst-slot matrices [slot, (t q)] and transpose
            dsl = dst_slot[eb]
            o = np.zeros((P, EB), NPBF)
            o[dsl, np.arange(n)] = 1.0
            ohT[:, b * EB:(b + 1) * EB] = o
            ot = o.reshape(P, NT, P).transpose(2, 1, 0).reshape(P, NT * P)
            oh[:, b * EB:(b + 1) * EB] = ot
        in_maps.append({"src_idx": sarr, "xselT": xsel, "ohT_in": ohT,
                        "oh_in": oh})

    def bc(v):
        return np.tile(np.asarray(v, np.float32)[None, :], (P, 1))

    consts = {
        "Wl1T": np.ascontiguousarray(weights["Wl1"].T).astype(NPBF),
        "Wr1T": np.ascontiguousarray(weights["Wr1"].T).astype(NPBF),
        "P1T": np.ascontiguousarray(weights["P1"].T).astype(NPBF),
        "brl1_bc": bc(weights["br1"] + weights["bl1"]),
        "pb1m1_bc": bc(weights["pb1"] - 1.0),
        "bias1_bc": bc(weights["bias1"] + weights["bl1"]),
        "att1_bc": bc(weights["att1"].reshape(-1)).astype(NPBF),
        "att1_rep": np.tile(bc(weights["att1"].reshape(-1)),
                            (1, NT)).astype(NPBF),
        "Wl2T": np.ascontiguousarray(weights["Wl2"].T).astype(NPBF),
        "Wr2T": np.ascontiguousarray(weights["Wr2"].T).astype(NPBF),
        "P2T": np.ascontiguousarray(weights["P2"].T).astype(NPBF),
        "brl2_bc": bc(weights["br2"] + weights["bl2"]),
        "pb2m1_bc": bc(weights["pb2"] - 1.0),
        "bias2_bc": bc(weights["bias2"] + weights["bl2"]),
        "att2_bc": bc(weights["att2"].reshape(-1)).astype(NPBF),
        "att2_rep": np.tile(bc(weights["att2"].reshape(-1)),
                            (1, NT)).astype(NPBF),
        "WfT": np.ascontiguousarray(weights["Wf"].T).astype(NPBF),
        "bf_bc": bc(weights["bf"]),
    }

    # permuted x, transposed: col (block*B+slot) = x[node]
    for c in range(NC):
        xp = np.zeros((NPC, IN_CH), np.float32)
        m = core_of == c
        xp[block_of[m] * B + slot_of[m]] = xf[m]
        im = in_maps[c]
        im["xT"] = np.ascontiguousarray(xp.T).astype(NPBF)
        im.update(consts)
    return in_maps, NT, (core_of, block_of, slot_of)


_CACHE = {}


def kernel(x, edge_index, Wl1, bl1, Wr1, br1, att1, bias1, P1, pb1,
           Wl2, bl2, Wr2, br2, att2, bias2, P2, pb2, Wf, bf):
    x = np.asarray(x)
    weights = dict(Wl1=np.asarray(Wl1), bl1=np.asarray(bl1),
                   Wr1=np.asarray(Wr1), br1=np.asarray(br1),
                   att1=np.asarray(att1), bias1=np.asarray(bias1),
                   P1=np.asarray(P1), pb1=np.asarray(pb1),
                   Wl2=np.asarray(Wl2), bl2=np.asarray(bl2),
                   Wr2=np.asarray(Wr2), br2=np.asarray(br2),
                   att2=np.asarray(att2), bias2=np.asarray(bias2),
                   P2=np.asarray(P2), pb2=np.asarray(pb2),
                   Wf=np.asarray(Wf), bf=np.asarray(bf))
    assert x.shape[0] == NC * NREAL, "hardcoded for the BrainGAT problem size"
    in_maps, NT, (core_of, block_of, slot_of) = prep_inputs(
        x, np.asarray(edge_index), weights)
    if NT not in _CACHE:
        _CACHE[NT] = build_program(NT)
    nc = _CACHE[NT]
    res = run_bass_kernel_spmd(nc, in_maps, list(range(NC)))
    full = np.concatenate([res.results[c]["out_own"] for c in range(NC)], 0)
    rows = core_of * NPC + block_of * B + slot_of
    return full[rows].astype(np.float32)


# revision 42
# speedup vs baseline: 1.0397x; 1.0397x over previous
"""BrainGAT (2x GATv2Conv + residuals + FC) on 8 Trainium2 NeuronCores.

Sharding: nodes partitioned across 8 cores via a load-balanced permutation
(160 bins of 128 slots, 125 real nodes each, in-degree balanced by snake
dealing); edges assigned to the bin owning their destination. Small weights
replicated.

Layer 1 needs no device gather at all: the host pre-gathers raw source
features per edge slot (XselT, [128ch x 128e] per tile, bf16) and the
device computes per-edge xl via XselT @ Wl1T on the Tensor engine -- the
same matmul slot the identity expansion used to occupy -- so the Q7 SWDGE
descriptor loop (~9ns/row) and the first AllGather disappear. The host
also supplies the dst one-hot matrices oh/ohT directly (DVE is_equal
builds removed). Layer 2 still gathers xl2 rows from the AllGathered
table, but as one 2048-idx single_packet=False gather per block spread
over 4 SWDGE queues (overlapping Q7 generation, ~3.9ns/idx).

Per-edge exp-denominator is fused into the numerator scatter (pexp rides
as 4 extra wptn columns through the same PSUM accumulation). Segment
softmax uses the exact no-max rewrite: alpha_e = exp(logit_e) /
(sum exp + exp(m_d)) with m_d the self-loop logit, so self-loops never
enter the edge lists. Linear biases bl ride through the softmax
(sum alpha = 1) and are folded into the xr-side and output biases.
"""
import numpy as np

import concourse.bass as bass
import concourse.bacc as bacc
import concourse.mybir as mybir
import concourse.tile as tile
from concourse.bass_utils import run_bass_kernel_spmd

f32 = mybir.dt.float32
bf16 = mybir.dt.bfloat16
i16 = mybir.dt.int16
NPBF = mybir.dt.np(bf16)
AF = mybir.ActivationFunctionType
ALU = mybir.AluOpType
PRELU = AF.Prelu

NC = 8
HEADS = 4
NEG_SLOPE = 0.2
P = 128
B = 128            # dst-block size (slots per bin)
NB = 20            # blocks per core
NPC = B * NB       # node slots per core (2560; 2500 real)
NREAL = 2500       # real nodes per core
NFULL = NC * NPC   # padded global table rows
IN_CH = 128
HC1 = 256
HC2 = 128
OUT_CH = 64


# ----------------------------------------------------------------------------
# device program
# ----------------------------------------------------------------------------

def build_program(NT, dbg=False):
    EB = NT * P               # edges per block (padded)
    IC = EB // 16             # idx cols per block (L2 gather)
    nc = bacc.Bacc("TRN2", target_bir_lowering=False, debug=False,
                   num_swdge_queues=4)

    def inp(name, shape, dt=f32):
        return nc.dram_tensor(name, shape, dt, kind="ExternalInput")

    xT = inp("xT", [IN_CH, NPC], bf16)
    xselT = inp("xselT", [P, NB * EB], bf16)      # [128ch, (b t q)]
    ohT_in = inp("ohT_in", [P, NB * EB], bf16)    # [slot p, (b t q)]
    oh_in = inp("oh_in", [P, NB * EB], bf16)      # [edge p, (b t q)]
    src_idx = inp("src_idx", [P, NB * IC], i16)   # L2 gather gids

    Wl1T = inp("Wl1T", [IN_CH, HC1], bf16); Wr1T = inp("Wr1T", [IN_CH, HC1], bf16)
    P1T = inp("P1T", [IN_CH, HC1], bf16)
    brl1_bc = inp("brl1_bc", [P, HC1])            # br1 + bl1
    pb1m1_bc = inp("pb1m1_bc", [P, HC1])          # pb1 - 1
    bias1_bc = inp("bias1_bc", [P, HC1])          # bias1 + bl1
    att1_bc = inp("att1_bc", [P, HC1], bf16)
    att1_rep = inp("att1_rep", [P, NT * HC1], bf16)

    Wl2T = inp("Wl2T", [HC1, HC2], bf16); Wr2T = inp("Wr2T", [HC1, HC2], bf16)
    P2T = inp("P2T", [HC1, HC2], bf16)
    brl2_bc = inp("brl2_bc", [P, HC2])            # br2 + bl2
    pb2m1_bc = inp("pb2m1_bc", [P, HC2])          # pb2 - 1
    bias2_bc = inp("bias2_bc", [P, HC2])          # bias2 + bl2
    att2_bc = inp("att2_bc", [P, HC2], bf16)
    att2_rep = inp("att2_rep", [P, NT * HC2], bf16)

    WfT = inp("WfT", [HC2, OUT_CH], bf16); bf_bc = inp("bf_bc", [P, OUT_CH])

    CHK = 5  # blocks per AllGather chunk
    NCHK = NB // CHK
    xl2_own = nc.dram_tensor("xl2_own", [NPC, HC2], bf16)
    # chunk-major full table: [chunk, core * (CHK*B) + row, ch] so each
    # chunk's AllGather output slice is contiguous
    xl2_full = nc.dram_tensor("xl2_full", [NCHK, NC * CHK * B, HC2], bf16,
                              addr_space="Shared")
    out_own = nc.dram_tensor("out_own", [NPC, OUT_CH], f32,
                             kind="ExternalOutput")
    if dbg:
        out_dbg = nc.dram_tensor("out_dbg", [P, 6 * HC1 + HC1 + HEADS], f32,
                                 kind="ExternalOutput")

    from concourse.masks import make_identity

    with tile.TileContext(nc) as tc:
        with (
            tc.tile_pool(name="const", bufs=1) as cp,
            tc.tile_pool(name="res", bufs=1) as rp,
            tc.tile_pool(name="work", bufs=2) as wp,
            tc.tile_pool(name="gath", bufs=4) as gp,
            tc.tile_pool(name="onehot", bufs=2) as op,
        ):
            # ---- constants
            identb = cp.tile([P, P], bf16)
            make_identity(nc, identb[:])
            with tc.tile_pool(name="pwarm", bufs=1, space="PSUM") as pw:
                warm = pw.tile([P, P], bf16)
                nc.tensor.transpose(out=warm[:], in_=identb[:],
                                    identity=identb[:])

            def load_const(t, shape):
                s = cp.tile(shape, t.dtype, tag=f"c_{t.name}")
                nc.sync.dma_start(out=s[:], in_=t[:, :])
                return s

            w1 = {k: load_const(v, [IN_CH, HC1])
                  for k, v in (("wl", Wl1T), ("wr", Wr1T), ("p", P1T))}
            c1b = {k: load_const(v, [P, HC1]) for k, v in (
                ("brl", brl1_bc), ("pbm1", pb1m1_bc), ("bias", bias1_bc))}
            c1b["attr"] = load_const(att1_rep, [P, NT * HC1])
            c1b["att"] = c1b["attr"][:, 0:HC1]

            def load_w2(t):
                s = cp.tile([P, 2 * HC2], t.dtype, tag=f"c_{t.name}")
                for c in range(2):
                    nc.sync.dma_start(out=s[:, c * HC2:(c + 1) * HC2],
                                      in_=t[c * P:(c + 1) * P, :])
                return s

            w2 = {k: load_w2(v) for k, v in (("wl", Wl2T), ("wr", Wr2T),
                                             ("p", P2T))}
            c2b = {k: load_const(v, [P, HC2]) for k, v in (
                ("brl", brl2_bc), ("pbm1", pb2m1_bc), ("bias", bias2_bc))}
            c2b["attr"] = load_const(att2_rep, [P, NT * HC2])
            c2b["att"] = c2b["attr"][:, 0:HC2]
            wf_sb = load_const(WfT, [HC2, OUT_CH])
            bf_sb = load_const(bf_bc, [P, OUT_CH])

            xT_sb = rp.tile([IN_CH, NPC], bf16)
            nc.sync.dma_start(out=xT_sb[:], in_=xT[:, :])
            sidx_sb = rp.tile([P, NB * IC], i16)
            nc.sync.dma_start(out=sidx_sb[:], in_=src_idx[:, :])

            # ---- persistent per-layer node tensors
            xl1_sb = rp.tile([P, NB * HC1], bf16)   # no bl1
            xr1_sb = rp.tile([P, NB * HC1], bf16)   # + br1 + bl1
            id1_sb = rp.tile([P, NB * HC1], bf16)
            em1_sb = rp.tile([P, NB * HEADS], f32)  # exp(self logit)
            h_sb = rp.tile([P, NB * HC1], bf16)
            hT_sb = rp.tile([P, NB * HC1], bf16)
            xl2_sb = rp.tile([P, NB * HC2], bf16)   # no bl2
            xr2_sb = rp.tile([P, NB * HC2], bf16)   # + br2 + bl2
            id2_sb = rp.tile([P, NB * HC2], bf16)
            em2_sb = rp.tile([P, NB * HEADS], f32)
            h2_sb = rp.tile([P, NB * HC2], bf16)

            # ---- N1: xl (no bias), xr (+brl), id, self-logit exp
            with tc.tile_pool(name="pn1", bufs=2, space="PSUM") as ps:
                for b in range(NB):
                    lhsT = xT_sb[:, b * B:(b + 1) * B]
                    pxl = ps.tile([P, HC1], f32, tag="pxl")
                    pxr = ps.tile([P, HC1], f32, tag="pxr")
                    pid = ps.tile([P, HC1], f32, tag="pid")
                    nc.tensor.matmul(out=pxl[:], lhsT=lhsT, rhs=w1["wl"][:],
                                     start=True, stop=True)
                    nc.tensor.matmul(out=pxr[:], lhsT=lhsT, rhs=w1["wr"][:],
                                     start=True, stop=True)
                    nc.tensor.matmul(out=pid[:], lhsT=lhsT, rhs=w1["p"][:],
                                     start=True, stop=True)
                    xl_t = xl1_sb[:, b * HC1:(b + 1) * HC1]
                    xr_t = xr1_sb[:, b * HC1:(b + 1) * HC1]
                    nc.scalar.activation(xl_t, pxl[:], AF.Copy)
                    nc.vector.tensor_add(out=xr_t, in0=pxr[:], in1=c1b["brl"][:])
                    nc.vector.tensor_add(out=id1_sb[:, b * HC1:(b + 1) * HC1],
                                         in0=pid[:], in1=c1b["pbm1"][:])
                    t0 = wp.tile([P, HC1], bf16, tag="t0")
                    nc.vector.tensor_add(out=t0[:], in0=xl_t, in1=xr_t)
                    nc.scalar.activation(t0[:], t0[:], PRELU, alpha=NEG_SLOPE)
                    nc.vector.tensor_mul(out=t0[:], in0=t0[:], in1=c1b["att"])
                    em_t = em1_sb[:, b * HEADS:(b + 1) * HEADS]
                    nc.vector.reduce_sum(
                        out=em_t,
                        in_=t0[:].rearrange("p (h c) -> p h c", h=HEADS),
                        axis=mybir.AxisListType.X)
                    nc.scalar.activation(em_t, em_t, AF.Exp)

            # ---- generic finalize: seg psum [P, hc+4] -> h_out block
            # xr_sub: when seg accumulated pexp*tt (tt = xl+xr_dst) instead of
            # pexp*xl, subtract xr_d * segE_d (exact: all edges share dst d).
            def finalize(b, segp, xl_sb, id_sb, em_sb, hout_sb, hc, consts,
                         xr_sub=None):
                CH = hc // HEADS
                e4 = em_sb[:, b * HEADS:(b + 1) * HEADS]
                den = wp.tile([P, HEADS], f32, tag="den")
                nc.vector.tensor_add(out=den[:], in0=segp[:, hc:hc + HEADS],
                                     in1=e4)
                rec = wp.tile([P, HEADS], f32, tag="rec")
                nc.vector.reciprocal(out=rec[:], in_=den[:])
                num = wp.tile([P, hc], f32, tag="num")
                nc.vector.tensor_tensor(
                    out=num[:].rearrange("p (h c) -> p h c", h=HEADS),
                    in0=xl_sb[:, b * hc:(b + 1) * hc]
                        .rearrange("p (h c) -> p h c", h=HEADS),
                    in1=e4.unsqueeze(2).to_broadcast([P, HEADS, CH]),
                    op=ALU.mult)
                nc.vector.tensor_add(out=num[:], in0=num[:], in1=segp[:, 0:hc])
                if xr_sub is not None:
                    t2 = wp.tile([P, hc], f32, tag="t2")
                    nc.vector.tensor_tensor(
                        out=t2[:].rearrange("p (h c) -> p h c", h=HEADS),
                        in0=xr_sub.rearrange("p (h c) -> p h c", h=HEADS),
                        in1=segp[:, hc:hc + HEADS].unsqueeze(2)
                            .to_broadcast([P, HEADS, CH]),
                        op=ALU.mult)
                    nc.vector.tensor_tensor(out=num[:], in0=num[:], in1=t2[:],
                                            op=ALU.subtract)
                nc.vector.tensor_tensor(
                    out=num[:].rearrange("p (h c) -> p h c", h=HEADS),
                    in0=num[:].rearrange("p (h c) -> p h c", h=HEADS),
                    in1=rec[:].unsqueeze(2).to_broadcast([P, HEADS, CH]),
                    op=ALU.mult)
                nc.vector.tensor_add(out=num[:], in0=num[:],
                                     in1=consts["bias"][:])
                # h = elu(num) + id = max(num,0) + (min(exp(num),1) + id)
                eu = wp.tile([P, hc], f32, tag="eu")
                nc.scalar.activation(eu[:], num[:], AF.Exp)
                t1 = wp.tile([P, hc], f32, tag="t1")
                nc.vector.scalar_tensor_tensor(
                    out=t1[:], in0=eu[:], scalar=1.0,
                    in1=id_sb[:, b * hc:(b + 1) * hc],
                    op0=ALU.min, op1=ALU.add)
                nc.vector.scalar_tensor_tensor(
                    out=hout_sb[:, b * hc:(b + 1) * hc], in0=num[:],
                    scalar=0.0, in1=t1[:], op0=ALU.max, op1=ALU.add)

            # ---- shared edge-phase tail: logits -> pexp -> wptn -> seg
            def edge_tail(b, tb, xlg, hc, consts, psa, att_eng=None):
                lgf = wp.tile([P, NT * HEADS], f32, tag="lgf")
                (att_eng or nc.vector).tensor_mul(
                    out=tb[:].rearrange("p t e -> p (t e)"),
                    in0=tb[:].rearrange("p t e -> p (t e)"),
                    in1=consts["attr"][:])
                nc.vector.reduce_sum(
                    out=lgf[:].rearrange("p (t h) -> p t h", t=NT),
                    in_=tb[:].rearrange("p t (h c) -> p t h c", h=HEADS),
                    axis=mybir.AxisListType.X)
                wptn = wp.tile([P, NT, hc + HEADS], bf16, tag="wptn")
                nc.scalar.activation(
                    wptn[:, :, hc:hc + HEADS],
                    lgf[:].rearrange("p (t h) -> p t h", t=NT), AF.Exp)
                nc.vector.tensor_tensor(
                    out=wptn[:, :, 0:hc]
                        .rearrange("p t (h c) -> p t h c", h=HEADS),
                    in0=xlg[:].rearrange("p t (h c) -> p t h c", h=HEADS),
                    in1=wptn[:, :, hc:hc + HEADS]
                        .unsqueeze(3).to_broadcast([P, NT, HEADS, hc // HEADS]),
                    op=ALU.mult)
                oh_sb = op.tile([P, NT, P], bf16, tag="oh")
                nc.sync.dma_start(
                    out=oh_sb[:].rearrange("p t q -> p (t q)"),
                    in_=oh_in[:, b * EB:(b + 1) * EB])
                seg = psa.tile([P, hc + HEADS], f32, tag="seg")
                for k in range(NT):
                    nc.tensor.matmul(out=seg[:], lhsT=oh_sb[:, k, :],
                                     rhs=wptn[:, k, :],
                                     start=(k == 0), stop=(k == NT - 1))
                return seg

            # ---- layer-1 edge phase (no gathers: XselT @ Wl1T on PE)
            # N2a (h transpose), N2b (xl2), and the chunked xl2 AllGather are
            # interleaved per block so their engine-queue entries come early.
            with (
                tc.tile_pool(name="pe1", bufs=2, space="PSUM") as psa,
                tc.tile_pool(name="pt1", bufs=4, space="PSUM") as pst,
            ):
                for b in range(NB):
                    xsel_sb = gp.tile([P, NT, P], bf16, tag="gsrc")
                    nc.sync.dma_start(
                        out=xsel_sb[:].rearrange("p t q -> p (t q)"),
                        in_=xselT[:, b * EB:(b + 1) * EB])
                    ohT_sb = op.tile([P, NT, P], bf16, tag="ohT")
                    nc.sync.dma_start(
                        out=ohT_sb[:].rearrange("p t q -> p (t q)"),
                        in_=ohT_in[:, b * EB:(b + 1) * EB])
                    xrb = xr1_sb[:, b * HC1:(b + 1) * HC1]
                    tts = wp.tile([P, NT, HC1], bf16, tag="tts")
                    tb = wp.tile([P, NT, HC1], bf16, tag="tb")
                    for j in range(NT // 2):
                        ttp = pst.tile([P, 2, HC1], f32, tag="ttp")
                        for i in range(2):
                            k = 2 * j + i
                            nc.tensor.matmul(out=ttp[:, i, :],
                                             lhsT=ohT_sb[:, k, :], rhs=xrb,
                                             start=True, stop=False)
                            nc.tensor.matmul(out=ttp[:, i, :],
                                             lhsT=xsel_sb[:, k, :],
                                             rhs=w1["wl"][:],
                                             start=False, stop=True)
                        nc.scalar.activation(
                            tts[:, 2 * j:2 * j + 2, :]
                                .rearrange("p t e -> p (t e)"),
                            ttp[:].rearrange("p t e -> p (t e)"), AF.Copy)
                        nc.scalar.activation(
                            tb[:, 2 * j:2 * j + 2, :]
                                .rearrange("p t e -> p (t e)"),
                            ttp[:].rearrange("p t e -> p (t e)"),
                            PRELU, alpha=NEG_SLOPE)
                    seg = edge_tail(b, tb, tts, HC1, c1b, psa)
                    if dbg and b == 0:
                        nc.gpsimd.dma_start(out=out_dbg.ap()[:, 0:HC1],
                                            in_=xl1_sb[:, 0:HC1])
                        nc.gpsimd.dma_start(out=out_dbg.ap()[:, HC1:2 * HC1],
                                            in_=xr1_sb[:, 0:HC1])
                        nc.gpsimd.dma_start(out=out_dbg.ap()[:, 2 * HC1:3 * HC1],
                                            in_=tts[:, 0, :])
                        nc.gpsimd.dma_start(out=out_dbg.ap()[:, 3 * HC1:4 * HC1],
                                            in_=tb[:, 0, :])
                        dbgt = wp.tile([P, HC1 + 2 * HEADS], f32, tag="dbg")
                        nc.vector.tensor_copy(out=dbgt[:, 0:HC1 + HEADS],
                                              in_=seg[:])
                        nc.vector.tensor_copy(
                            out=dbgt[:, HC1 + HEADS:HC1 + 2 * HEADS],
                            in_=em1_sb[:, 0:HEADS])
                        nc.sync.dma_start(
                            out=out_dbg.ap()[:, 4 * HC1:5 * HC1 + 2 * HEADS],
                            in_=dbgt[:])
                    finalize(b, seg, xl1_sb, id1_sb, em1_sb, h_sb, HC1, c1b,
                             xr_sub=xrb)
                    if dbg and b == 0:
                        nc.gpsimd.dma_start(
                            out=out_dbg.ap()[:, 6 * HC1:7 * HC1],
                            in_=h_sb[:, 0:HC1])
                    # N2a: transpose h[b] via HWDGE X-bar
                    for c in range(2):
                        sl = slice(b * HC1 + c * P, b * HC1 + (c + 1) * P)
                        nc.sync.dma_start(out=hT_sb[:, sl], in_=h_sb[:, sl],
                                          transpose=True)
                    # N2b: xl2[b] (no bias)
                    pxl = psa.tile([P, HC2], f32, tag="p2xl")
                    for c in range(2):
                        nc.tensor.matmul(
                            out=pxl[:],
                            lhsT=hT_sb[:, b * HC1 + c * P:b * HC1 + (c + 1) * P],
                            rhs=w2["wl"][:, c * HC2:(c + 1) * HC2],
                            start=(c == 0), stop=(c == 1))
                    nc.scalar.activation(xl2_sb[:, b * HC2:(b + 1) * HC2],
                                         pxl[:], AF.Copy)
                    if b % CHK == CHK - 1:
                        c0 = b // CHK
                        nc.sync.dma_start(
                            out=xl2_own.ap()[c0 * CHK * B:(c0 + 1) * CHK * B, :]
                                .rearrange("(b p) c -> p b c", p=B),
                            in_=xl2_sb[:, c0 * CHK * HC2:(c0 + 1) * CHK * HC2]
                                .rearrange("p (b c) -> p b c", b=CHK))
                        nc.gpsimd.collective_compute(
                            "AllGather", ALU.bypass,
                            replica_groups=[list(range(NC))],
                            ins=[xl2_own.ap()
                                 [c0 * CHK * B:(c0 + 1) * CHK * B, :].opt()],
                            outs=[xl2_full.ap()[c0, :, :].opt()])
            with tc.tile_pool(name="pn2c", bufs=2, space="PSUM") as ps:
                for b in range(NB):
                    pxr = ps.tile([P, HC2], f32, tag="p2xr")
                    pid = ps.tile([P, HC2], f32, tag="p2id")
                    for c in range(2):
                        lhsT = hT_sb[:, b * HC1 + c * P:b * HC1 + (c + 1) * P]
                        st, sp = (c == 0), (c == 1)
                        cs = slice(c * HC2, (c + 1) * HC2)
                        nc.tensor.matmul(out=pxr[:], lhsT=lhsT,
                                         rhs=w2["wr"][:, cs], start=st, stop=sp)
                        nc.tensor.matmul(out=pid[:], lhsT=lhsT,
                                         rhs=w2["p"][:, cs], start=st, stop=sp)
                    xr_t = xr2_sb[:, b * HC2:(b + 1) * HC2]
                    nc.vector.tensor_add(out=xr_t, in0=pxr[:], in1=c2b["brl"][:])
                    nc.vector.tensor_add(out=id2_sb[:, b * HC2:(b + 1) * HC2],
                                         in0=pid[:], in1=c2b["pbm1"][:])
                    t0 = wp.tile([P, HC2], bf16, tag="t02")
                    nc.vector.tensor_add(out=t0[:],
                                         in0=xl2_sb[:, b * HC2:(b + 1) * HC2],
                                         in1=xr_t)
                    nc.scalar.activation(t0[:], t0[:], PRELU, alpha=NEG_SLOPE)
                    nc.vector.tensor_mul(out=t0[:], in0=t0[:], in1=c2b["att"])
                    em_t = em2_sb[:, b * HEADS:(b + 1) * HEADS]
                    nc.vector.reduce_sum(
                        out=em_t,
                        in_=t0[:].rearrange("p (h c) -> p h c", h=HEADS),
                        axis=mybir.AxisListType.X)
                    nc.scalar.activation(em_t, em_t, AF.Exp)

            # ---- layer-2 edge phase (gathered xlg; tt = ohT@xr2 + xlg on DVE)
            with (
                tc.tile_pool(name="pe2", bufs=2, space="PSUM") as psa,
                tc.tile_pool(name="pt2", bufs=4, space="PSUM") as pst,
            ):
                for b in range(NB):
                    xlg = gp.tile([P, NT, HC2], bf16, tag="gsrc")
                    nc.gpsimd.dma_gather(
                        xlg[:], xl2_full.ap().rearrange("a b c -> (a b) c"),
                        sidx_sb[:, b * IC:(b + 1) * IC],
                        EB, EB, HC2, single_packet=False, queue_num=b % 4)
                    ohT_sb = op.tile([P, NT, P], bf16, tag="ohT")
                    nc.sync.dma_start(
                        out=ohT_sb[:].rearrange("p t q -> p (t q)"),
                        in_=ohT_in[:, b * EB:(b + 1) * EB])
                    xrb = xr2_sb[:, b * HC2:(b + 1) * HC2]
                    tb = wp.tile([P, NT, HC2], bf16, tag="tb2")
                    for j in range(NT // 2):
                        ttp = pst.tile([P, 2, HC2], f32, tag="ttp2")
                        for i in range(2):
                            k = 2 * j + i
                            nc.tensor.matmul(out=ttp[:, i, :],
                                             lhsT=ohT_sb[:, k, :], rhs=xrb,
                                             start=True, stop=True)
                        nc.vector.tensor_add(
                            out=tb[:, 2 * j:2 * j + 2, :]
                                .rearrange("p t e -> p (t e)"),
                            in0=ttp[:].rearrange("p t e -> p (t e)"),
                            in1=xlg[:, 2 * j:2 * j + 2, :]
                                .rearrange("p t e -> p (t e)"))
                        nc.scalar.activation(
                            tb[:, 2 * j:2 * j + 2, :]
                                .rearrange("p t e -> p (t e)"),
                            tb[:, 2 * j:2 * j + 2, :]
                                .rearrange("p t e -> p (t e)"),
                            PRELU, alpha=NEG_SLOPE)
                    seg = edge_tail(b, tb, xlg, HC2, c2b, psa)
                    finalize(b, seg, xl2_sb, id2_sb, em2_sb, h2_sb, HC2, c2b)
                    # FC for block b
                    h2T = wp.tile([P, P], bf16, tag="fcT")
                    nc.sync.dma_start(out=h2T[:],
                                      in_=h2_sb[:, b * HC2:(b + 1) * HC2],
                                      transpose=True)
                    pf = psa.tile([P, OUT_CH], f32, tag="fc_out")
                    nc.tensor.matmul(out=pf[:], lhsT=h2T[:], rhs=wf_sb[:],
                                     start=True, stop=True)
                    ob = wp.tile([P, OUT_CH], f32, tag="fc_ob")
                    nc.vector.tensor_add(out=ob[:], in0=pf[:], in1=bf_sb[:])
                    nc.sync.dma_start(out=out_own.ap()[b * B:(b + 1) * B, :],
                                      in_=ob[:])
    nc.compile()
    return nc


# ----------------------------------------------------------------------------
# host-side sharding / input prep
# ----------------------------------------------------------------------------

def balanced_assignment(dst):
    """Snake-deal nodes (by in-degree desc) into NC*NB bins of NPC/NB slots."""
    N = NC * NREAL
    nbins = NC * NB
    rounds = N // nbins  # 125
    deg = np.bincount(dst, minlength=N)
    order = np.argsort(-deg, kind="stable")
    bin_of = np.empty(N, np.int32)
    slot_of = np.empty(N, np.int32)
    cols = np.arange(nbins)
    for r in range(rounds):
        nodes = order[r * nbins:(r + 1) * nbins]
        c = cols if r % 2 == 0 else cols[::-1]
        bin_of[nodes] = c
        slot_of[nodes] = r
    return bin_of, slot_of


def wrap_idx(vals, EB):
    """[EB] int -> [128, EB//16] int16 wrapped in 16 partitions, replicated."""
    w = np.zeros((16, EB // 16), np.int16)
    w[np.arange(EB) % 16, np.arange(EB) // 16] = vals.astype(np.int16)
    return np.tile(w, (8, 1))


def prep_inputs(x, edge_index, weights):
    src = np.asarray(edge_index[0], dtype=np.int64)
    dst = np.asarray(edge_index[1], dtype=np.int64)

    bin_of, slot_of = balanced_assignment(dst)
    core_of = bin_of // NB
    block_of = bin_of % NB
    # row in the chunk-major L2 full table [chunk, core*(CHK*B)+row, ch]
    CHK = 5
    gid = ((block_of // CHK) * (NC * CHK * B) + core_of * (CHK * B)
           + (block_of % CHK) * B + slot_of)

    ebin = bin_of[dst]
    counts = np.bincount(ebin, minlength=NC * NB)
    NT = max(1, int(np.ceil(counts.max() / P)))
    EB = NT * P
    IC = EB // 16

    eorder = np.argsort(ebin, kind="stable")
    offs = np.zeros(NC * NB + 1, np.int64)
    np.cumsum(counts, out=offs[1:])

    src_gid = gid[src]
    dst_slot = slot_of[dst]
    xf = np.asarray(x, np.float32)
    xT_bf = np.ascontiguousarray(xf.T).astype(NPBF)  # [128, N] by node id

    in_maps = []
    eye = np.eye(P, dtype=NPBF)
    for c in range(NC):
        sarr = np.zeros((P, NB * IC), np.int16)
        xsel = np.zeros((P, NB * EB), NPBF)
        ohT = np.zeros((P, NB * EB), NPBF)
        oh = np.zeros((P, NB * EB), NPBF)
        for b in range(NB):
            bi = c * NB + b
            eb = eorder[offs[bi]:offs[bi + 1]]
            n = len(eb)
            sv = np.zeros(EB, np.int64); sv[:n] = src_gid[eb]
            sarr[:, b * IC:(b + 1) * IC] = wrap_idx(sv, EB)
            # per-edge source features [128ch, EB]
            xsel[:, b * EB:b * EB + n] = xT_bf[:, src[eb]]
            # one-hot dst-slot matrices [slot, (t q)] and transpose
            dsl = dst_slot[eb]
            o = np.zeros((P, EB), NPBF)
            o[dsl, np.arange(n)] = 1.0
            ohT[:, b * EB:(b + 1) * EB] = o
            ot = o.reshape(P, NT, P).transpose(2, 1, 0).reshape(P, NT * P)
            oh[:, b * EB:(b + 1) * EB] = ot
        in_maps.append({"src_idx": sarr, "xselT": xsel, "ohT_in": ohT,
                        "oh_in": oh})

    def bc(v):
        return np.tile(np.asarray(v, np.float32)[None, :], (P, 1))

    consts = {
        "Wl1T": np.ascontiguousarray(weights["Wl1"].T).astype(NPBF),
        "Wr1T": np.ascontiguousarray(weights["Wr1"].T).astype(NPBF),
        "P1T": np.ascontiguousarray(weights["P1"].T).astype(NPBF),
        "brl1_bc": bc(weights["br1"] + weights["bl1"]),
        "pb1m1_bc": bc(weights["pb1"] - 1.0),
        "bias1_bc": bc(weights["bias1"] + weights["bl1"]),
        "att1_bc": bc(weights["att1"].reshape(-1)).astype(NPBF),
        "att1_rep": np.tile(bc(weights["att1"].reshape(-1)),
                            (1, NT)).astype(NPBF),
        "Wl2T": np.ascontiguousarray(weights["Wl2"].T).astype(NPBF),
        "Wr2T": np.ascontiguousarray(weights["Wr2"].T).astype(NPBF),
        "P2T": np.ascontiguousarray(weights["P2"].T).astype(NPBF),
        "brl2_bc": bc(weights["br2"] + weights["bl2"]),
        "pb2m1_bc": bc(weights["pb2"] - 1.0),
        "bias2_bc": bc(weights["bias2"] + weights["bl2"]),
        "att2_bc": bc(weights["att2"].reshape(-1)).astype(NPBF),
        "att2_rep": np.tile(bc(weights["att2"].reshape(-1)),
                            (1, NT)).astype(NPBF),
        "WfT": np.ascontiguousarray(weights["Wf"].T).astype(NPBF),
        "bf_bc": bc(weights["bf"]),
    }

    # permuted x, transposed: col (block*B+slot) = x[node]
    for c in range(NC):
        xp = np.zeros((NPC, IN_CH), np.float32)
        m = core_of == c
        xp[block_of[m] * B + slot_of[m]] = xf[m]
        im = in_maps[c]
        im["xT"] = np.ascontiguousarray(xp.T).astype(NPBF)
        im.update(consts)
    return in_maps, NT, (core_of, block_of, slot_of)


_CACHE = {}


def kernel(x, edge_index, Wl1, bl1, Wr1, br1, att1, bias1, P1, pb1,
           Wl2, bl2, Wr2, br2, att2, bias2, P2, pb2, Wf, bf):
    x = np.asarray(x)
    weights = dict(Wl1=np.asarray(Wl1), bl1=np.asarray(bl1),
                   Wr1=np.asarray(Wr1), br1=np.asarray(br1),
                   att1=np.asarray(att1), bias1=np.asarray(bias1),
                   P1=np.asarray(P1), pb1=np.asarray(pb1),
                   Wl2=np.asarray(Wl2), bl2=np.asarray(bl2),
                   Wr2=np.asarray(Wr2), br2=np.asarray(br2),
                   att2=np.asarray(att2), bias2=np.asarray(bias2),
                   P2=np.asarray(P2), pb2=np.asarray(pb2),
                   Wf=np.asarray(Wf), bf=np.asarray(bf))
    assert x.shape[0] == NC * NREAL, "hardcoded for the BrainGAT problem size"
    in_maps, NT, (core_of, block_of, slot_of) = prep_inputs(
        x, np.asarray(edge_index), weights)
    if NT not in _CACHE:
        _CACHE[NT] = build_program(NT)
    nc = _CACHE[NT]
    res = run_bass_kernel_spmd(nc, in_maps, list(range(NC)))
    full = np.concatenate([res.results[c]["out_own"] for c in range(NC)], 0)
    rows = core_of * NPC + block_of * B + slot_of
    return full[rows].astype(np.float32)
